# revision 55
# baseline (speedup 1.0000x reference)
"""Trainium2 Bass kernel for KG-enhanced embedding model (gnn_message_passing).

Computes, for full inputs:
    inputs_embeds = word_embedding[input_ids]                       # [B,S,H] gather
    h   = relu(entity_embeddings @ W1 + b1)                         # [B,E,MLP_HID]
    ent = h @ W2 + b2                                               # [B,E,H]
    out = inputs_embeds + einsum('bes,beh->bsh', entity_mask, ent)  # masked scatter-add

Sharding: data-parallel over batch B=32 -> 4 examples per core on 8 cores.
Weights and the vocab table are replicated.

The harness gate is rel_err < 2e-2, so everything runs in single bf16
(no hi/lo split): the vocab table is cast to bf16 on host (halves the
gather read), weights/mask are bf16, and the output is stored bf16 and
upcast to f32 on host (halves the store write). Measured rel err ~2e-3.

The vocab gather runs as 16 x 128-row mainline-SWDGE indirect DMAs:
the Q7 descriptor generator paces them at ~1.4us/chunk, which is the
kernel's critical stream (measured facts: dma_gather's multi-queue
desc-gen is ~2.3x faster per row but requires a ~13us Q7 library load
during which NO descriptors can be generated, so it always loses;
multi-row offset APs on indirect DMA corrupt data on HW; fp8 matmuls
do NOT double-pump the PE; the PE stays at 1.2GHz pstate regardless
of warmup).

W2 loads in 4 pieces alternating both HWDGE rings so stage2 k-chunks
unblock progressively under HBM contention. Output rides grouped
stores of [128, n*768] (partition-major DRAM layout, host
re-transposes) so each HWDGE descriptor moves >=3KB; the last 4
chunks ship as 2-chunk stores to cut the drain tail.

Shapes (hardcoded): V=30522, H=768, B=32, S=512, E=8, KG=100, MH=1000.
"""

import os
import numpy as np
from contextlib import ExitStack

V, H = 30522, 768
B, S, E = 32, 512, 8
KG, MH = 100, 1000
NCORES = 8
BPC = B // NCORES              # examples per core = 4
TOK = BPC * S                  # tokens per core = 2048
NCH = TOK // 128               # 128-token chunks per core = 16
KCH = 8                        # K chunks of 128 for the 1000-dim contraction
NE = BPC * E                   # entities per core = 32
GQ = 4                         # gather instructions per core
GCH = NCH // GQ                # 128-token chunks per gather = 4
GROWS = TOK // GQ              # rows per gather = 512

_PROGRAM = None


def _maybe_enable_profiling():
    """Optional NTFF profiling (KERNEL_PROFILE=1): shim antenv.axon_hooks."""
    if os.environ.get("KERNEL_PROFILE") != "1":
        return False
    import sys, types
    try:
        from antenv.axon_hooks import get_axon_ntff_profile_hook  # noqa: F401
        return True
    except ImportError:
        pass
    try:
        from trn_agent_boot.trn_boot import _ntff_profile_via_ctypes
        import antenv
        hook = _ntff_profile_via_ctypes("/opt/axon/libaxon_pjrt.so")
        m = types.ModuleType("antenv.axon_hooks")
        m.get_axon_ntff_profile_hook = lambda: hook
        m.set_axon_ntff_profile_hook = lambda h: None
        sys.modules["antenv.axon_hooks"] = m
        antenv.axon_hooks = m
        return True
    except Exception:
        return False


def _build_program():
    import concourse.bacc as bacc
    import concourse.tile as tile
    from concourse import bass, mybir

    f32 = mybir.dt.float32
    bf16 = mybir.dt.bfloat16
    i32 = mybir.dt.int32
    RELU = mybir.ActivationFunctionType.Relu

    nc = bacc.Bacc("TRN2", target_bir_lowering=False, debug=False)

    # idsT[p, g] = ids[g*128 + p] (indirect-DMA row offsets per 128-chunk)
    idx_ap = nc.dram_tensor("idsT", [128, NCH], i32, kind="ExternalInput").ap()
    web_ap = nc.dram_tensor("web", [V, H], bf16, kind="ExternalInput").ap()
    # w1ee packs [W1 | eeT]: [KG, MH + NE]
    w1ee_ap = nc.dram_tensor("w1ee", [KG, MH + NE], bf16, kind="ExternalInput").ap()
    b1c_ap = nc.dram_tensor("b1colT", [128, KCH], f32, kind="ExternalInput").ap()
    # w2p chunk-major: w2p[p, k*H+j] = W2[k*128+p, j] (rows 1000..1023 zero)
    w2_ap = nc.dram_tensor("w2p", [128, KCH * H], bf16, kind="ExternalInput").ap()
    # b2o [1, H+NE]: [b2 | ones] -> K=1 bias matmul
    b2o_ap = nc.dram_tensor("b2o", [1, H + NE], bf16, kind="ExternalInput").ap()
    maskT_ap = nc.dram_tensor("maskT", [NE, TOK], bf16, kind="ExternalInput").ap()
    # out[p, g*H:(g+1)*H] = token (g*128+p); host re-transposes
    out_ap = nc.dram_tensor("out", [128, NCH * H], bf16, kind="ExternalOutput").ap()

    with tile.TileContext(nc) as tc, ExitStack() as ctx:
        const = ctx.enter_context(tc.tile_pool(name="const", bufs=1))
        psA = ctx.enter_context(tc.tile_pool(name="psA", bufs=2, space="PSUM"))
        psE = ctx.enter_context(tc.tile_pool(name="psE", bufs=1, space="PSUM"))
        psC = ctx.enter_context(tc.tile_pool(name="psC", bufs=2, space="PSUM"))
        gpool = ctx.enter_context(tc.tile_pool(name="gath", bufs=NCH))

        # ---- loads -----------------------------------------------------------
        # sync HWDGE:   idx (gathers need it first) + w1ee + w2 1st half
        # scalar HWDGE: small weights + w2 2nd half + mask
        # w1ee ahead of idx: the PE/MLP chain (entS -> vector-add start) is
        # the binding stream; the Q7 gather stream has ~4us of slack, so
        # delaying the gather start ~0.6us to start stage1 earlier nets out.
        w1ee_sb = const.tile([KG, MH + NE], bf16)
        nc.sync.dma_start(w1ee_sb[:], w1ee_ap[:])
        idx_sb = const.tile([128, NCH], i32)
        nc.sync.dma_start(idx_sb[:], idx_ap[:])
        w1_sb = w1ee_sb[:, :MH]
        ee_sb = w1ee_sb[:, MH : MH + NE]
        b1_col = const.tile([128, KCH], f32)
        nc.scalar.dma_start(b1_col[:], b1c_ap[:])
        b2o_sb = const.tile([1, H + NE], bf16)
        nc.scalar.dma_start(b2o_sb[:], b2o_ap[:])
        # w2 in 2-chunk pieces alternating rings so stage2 k-chunks unblock
        # progressively instead of waiting for the whole 1.5MB under HBM
        # contention with the gather stream.
        w2_sb = const.tile([128, KCH * H], bf16)
        for piece in range(4):
            lo = piece * 2 * H
            hi = lo + 2 * H
            eng = nc.sync if piece % 2 == 0 else nc.scalar
            eng.dma_start(w2_sb[:, lo:hi], w2_ap[:, lo:hi])
        maskT_sb = const.tile([NE, TOK], bf16)
        nc.scalar.dma_start(maskT_sb[:], maskT_ap[:])

        # ---- vocab gathers: 16 x 128 rows via indirect DMA. Mainline-SWDGE
        # only: any dma_gather would force a ~13us Q7 library load during
        # which NO descriptors (even mainline) can be generated.
        gts = []
        for g in range(NCH):
            gt = gpool.tile([128, H], bf16, tag="gt")
            nc.gpsimd.indirect_dma_start(
                out=gt[:],
                out_offset=None,
                in_=web_ap[:],
                in_offset=bass.IndirectOffsetOnAxis(ap=idx_sb[:, g : g + 1], axis=0),
            )
            gts.append(gt)

        # ---- MLP stage 1: hT[k*128+p, e] = relu(W1.T @ ee.T + b1) -----------
        hT = const.tile([128, KCH, NE], bf16)
        nc.vector.memset(hT[96:128, KCH - 1, :], 0.0)
        for k in range(KCH):
            mw = 128 if k < KCH - 1 else MH - 128 * (KCH - 1)  # 104 in last
            ps = psA.tile([128, NE], f32, tag="ps")
            nc.tensor.matmul(
                out=ps[:mw, :],
                lhsT=w1_sb[:, k * 128 : k * 128 + mw],
                rhs=ee_sb[:],
                start=True,
                stop=True,
            )
            nc.scalar.activation(
                out=hT[:mw, k, :],
                in_=ps[:mw, :],
                func=RELU,
                bias=b1_col[:mw, k : k + 1],
            )

        # ---- MLP stage 2: ent = hT.T @ W2 + b2 ------------------------------
        # b2 enters the PSUM accumulation as a K=1 matmul of ones.T @ b2.
        entp = psE.tile([NE, H], f32)
        NGROUPS = ((0, 512), (512, H))
        for n0, n1 in NGROUPS:
            nc.tensor.matmul(
                out=entp[:, n0:n1],
                lhsT=b2o_sb[:, H : H + NE],
                rhs=b2o_sb[:, n0:n1],
                start=True,
                stop=False,
            )
        for k in range(KCH):
            for n0, n1 in NGROUPS:
                nc.tensor.matmul(
                    out=entp[:, n0:n1],
                    lhsT=hT[:, k, :],
                    rhs=w2_sb[:, k * H + n0 : k * H + n1],
                    start=False,
                    stop=(k == KCH - 1),
                )
        entS = const.tile([NE, H], bf16)
        nc.scalar.copy(entS[:], entp[:])

        # ---- main loop: scatter-matmul, add, grouped store ------------------
        # Vector drains every PSUM tile (measured best: a second drain lane
        # via scalar-ACT copies or gpsimd adds always loses to scheduler and
        # ring-order effects). Stores group 4 chunks; the last 4 ship as
        # 2-chunk stores to cut the drain tail.
        opool = ctx.enter_context(tc.tile_pool(name="outp", bufs=GQ))
        ot_q = None
        for g in range(NCH):
            if g % GCH == 0:
                ot_q = opool.tile([128, GCH, H], bf16, tag="ot")
            sc = psC.tile([128, H], f32, tag="sc")
            for n0, n1 in NGROUPS:
                nc.tensor.matmul(
                    out=sc[:, n0:n1],
                    lhsT=maskT_sb[:, g * 128 : (g + 1) * 128],
                    rhs=entS[:, n0:n1],
                    start=True,
                    stop=True,
                )
            nc.vector.tensor_add(ot_q[:, g % GCH, :], gts[g][:], sc[:])
            if g == 13 or g == 15:
                st_eng = nc.sync if g == 13 else nc.scalar
                st_eng.dma_start(
                    out_ap[:, (g - 1) * H : (g + 1) * H],
                    ot_q[:, (g - 1) % GCH : g % GCH + 1, :],
                )
            elif (g + 1) % GCH == 0:
                q = g // GCH
                st_eng = nc.sync if q % 2 == 0 else nc.scalar
                st_eng.dma_start(
                    out_ap[:, q * GCH * H : (q + 1) * GCH * H], ot_q[:]
                )

    nc.compile()
    return nc


def _get_program():
    global _PROGRAM
    if _PROGRAM is None:
        _PROGRAM = _build_program()
    return _PROGRAM


def _prep_shards(inputs):
    import ml_dtypes

    bf = ml_dtypes.bfloat16
    ids = np.asarray(inputs["input_ids"]).astype(np.int32)
    ee = np.asarray(inputs["entity_embeddings"], dtype=np.float32)
    mask = np.asarray(inputs["entity_mask"], dtype=np.float32)
    we = np.asarray(inputs["word_embedding"], dtype=np.float32)
    W1 = np.asarray(inputs["W1"], dtype=np.float32)
    b1 = np.asarray(inputs["b1"], dtype=np.float32)
    W2 = np.asarray(inputs["W2"], dtype=np.float32)
    b2 = np.asarray(inputs["b2"], dtype=np.float32)

    web = np.ascontiguousarray(we.astype(bf))
    w1b = W1.astype(bf)
    w2_pad = np.concatenate([W2, np.zeros((KCH * 128 - MH, H), np.float32)], 0)
    w2p = np.ascontiguousarray(
        w2_pad.reshape(KCH, 128, H).transpose(1, 0, 2).reshape(128, KCH * H).astype(bf)
    )
    b2o = np.ascontiguousarray(
        np.concatenate([b2[None, :], np.ones((1, NE), np.float32)], 1).astype(bf)
    )  # [1, H+NE]
    b1pad = np.concatenate([b1, np.zeros(KCH * 128 - MH, np.float32)])
    b1colT = np.ascontiguousarray(b1pad.reshape(KCH, 128).T)  # [128, KCH]

    in_maps = []
    for i in range(NCORES):
        sl = slice(BPC * i, BPC * (i + 1))
        ids_c = ids[sl].reshape(-1)  # [TOK]
        idsT = np.ascontiguousarray(ids_c.reshape(NCH, 128).T)  # [128, NCH]
        # tail tokens (chunks 10..15) wrapped mod 16 for dma_gather:
        # idx16t[p, s] = ids_c[1280 + s*16 + (p % 16)]
        tail = ids_c[10 * 128 :].reshape(-1, 16)  # [48, 16]
        idx16t = np.tile(tail.T, (8, 1)).astype(np.int16)  # [128, 48]
        eeT = ee[sl].reshape(NE, KG).T.astype(bf)  # [KG, NE]
        w1ee = np.ascontiguousarray(np.concatenate([w1b, eeT], 1))
        # block-diagonal [NE, TOK] mask (0/1 values: exact in bf16)
        maskT = np.zeros((NE, TOK), np.float32)
        for b in range(BPC):
            maskT[b * E : (b + 1) * E, b * S : (b + 1) * S] = mask[BPC * i + b]
        in_maps.append(
            {
                "idsT": idsT,
                "idx16t": np.ascontiguousarray(idx16t),
                "web": web,
                "w1ee": w1ee,
                "b1colT": b1colT,
                "w2p": w2p,
                "b2o": b2o,
                "maskT": np.ascontiguousarray(maskT.astype(bf)),
            }
        )
    return in_maps


def kernel(**inputs) -> np.ndarray:
    from concourse.bass_utils import run_bass_kernel_spmd

    trace = _maybe_enable_profiling()
    nc = _get_program()
    in_maps = _prep_shards(inputs)
    res = run_bass_kernel_spmd(
        nc, in_maps, core_ids=list(range(NCORES)), trace=trace
    )
    if trace and res.exec_time_ns is not None:
        print(f"HW exec time: {res.exec_time_ns} ns")
    # out[p, g*H:(g+1)*H] = token (g*128+p): re-transpose per core
    outs = []
    for i in range(NCORES):
        o = np.asarray(res.results[i]["out"]).astype(np.float32)
        o = o.reshape(128, NCH, H).transpose(1, 0, 2).reshape(BPC, S, H)
        outs.append(o)
    return np.concatenate(outs, 0)


if __name__ == "__main__":
    rng = np.random.default_rng(0)
    inputs = {
        "input_ids": rng.integers(0, V, (B, S)).astype(np.int32),
        "entity_embeddings": rng.standard_normal((B, E, KG), dtype=np.float32),
        "entity_mask": (rng.random((B, E, S)) < 0.02).astype(np.float32),
        "word_embedding": rng.standard_normal((V, H), dtype=np.float32) * 0.02,
        "W1": rng.standard_normal((KG, MH), dtype=np.float32) * 0.02,
        "b1": np.zeros(MH, np.float32),
        "W2": rng.standard_normal((MH, H), dtype=np.float32) * 0.02,
        "b2": np.zeros(H, np.float32),
    }
    out = kernel(**inputs)
    ref = inputs["word_embedding"][inputs["input_ids"]] + np.einsum(
        "bes,beh->bsh",
        inputs["entity_mask"],
        np.maximum(
            inputs["entity_embeddings"] @ inputs["W1"] + inputs["b1"], 0.0
        )
        @ inputs["W2"]
        + inputs["b2"],
    )
    err = np.abs(out - ref).max() / max(np.abs(ref).max(), 1e-12)
    print("self-check rel err:", err)


# revision 56
# speedup vs baseline: 1.0487x; 1.0487x over previous
"""Trainium2 Bass kernel for KG-enhanced embedding model (gnn_message_passing).

Computes, for full inputs:
    inputs_embeds = word_embedding[input_ids]                       # [B,S,H] gather
    h   = relu(entity_embeddings @ W1 + b1)                         # [B,E,MLP_HID]
    ent = h @ W2 + b2                                               # [B,E,H]
    out = inputs_embeds + einsum('bes,beh->bsh', entity_mask, ent)  # masked scatter-add

Sharding: data-parallel over batch B=32 -> 4 examples per core on 8 cores.
Weights and the vocab table are replicated.

The harness gate is rel_err < 2e-2, so everything runs in single bf16
(no hi/lo split): the vocab table is cast to bf16 on host (halves the
gather read), weights/mask are bf16, and the output is stored bf16 and
upcast to f32 on host (halves the store write). Measured rel err ~2e-3.

The vocab gather runs as 16 x 128-row mainline-SWDGE indirect DMAs:
the Q7 descriptor generator paces them at ~1.4us/chunk, which is the
kernel's critical stream (measured facts: dma_gather's multi-queue
desc-gen is ~2.3x faster per row but requires a ~13us Q7 library load
during which NO descriptors can be generated, so it always loses;
multi-row offset APs on indirect DMA corrupt data on HW; fp8 matmuls
do NOT double-pump the PE; the PE stays at 1.2GHz pstate regardless
of warmup).

W2 loads in 4 pieces alternating both HWDGE rings so stage2 k-chunks
unblock progressively under HBM contention. Output rides grouped
stores of [128, n*768] (partition-major DRAM layout, host
re-transposes) so each HWDGE descriptor moves >=3KB; the last 4
chunks ship as 2-chunk stores to cut the drain tail.

Shapes (hardcoded): V=30522, H=768, B=32, S=512, E=8, KG=100, MH=1000.
"""

import os
import numpy as np
from contextlib import ExitStack

V, H = 30522, 768
B, S, E = 32, 512, 8
KG, MH = 100, 1000
NCORES = 8
BPC = B // NCORES              # examples per core = 4
TOK = BPC * S                  # tokens per core = 2048
NCH = TOK // 128               # 128-token chunks per core = 16
KCH = 8                        # K chunks of 128 for the 1000-dim contraction
NE = BPC * E                   # entities per core = 32
GQ = 4                         # gather instructions per core
GCH = NCH // GQ                # 128-token chunks per gather = 4
GROWS = TOK // GQ              # rows per gather = 512

_PROGRAM = None


def _maybe_enable_profiling():
    """Optional NTFF profiling (KERNEL_PROFILE=1): shim antenv.axon_hooks."""
    if os.environ.get("KERNEL_PROFILE") != "1":
        return False
    import sys, types
    try:
        from antenv.axon_hooks import get_axon_ntff_profile_hook  # noqa: F401
        return True
    except ImportError:
        pass
    try:
        from trn_agent_boot.trn_boot import _ntff_profile_via_ctypes
        import antenv
        hook = _ntff_profile_via_ctypes("/opt/axon/libaxon_pjrt.so")
        m = types.ModuleType("antenv.axon_hooks")
        m.get_axon_ntff_profile_hook = lambda: hook
        m.set_axon_ntff_profile_hook = lambda h: None
        sys.modules["antenv.axon_hooks"] = m
        antenv.axon_hooks = m
        return True
    except Exception:
        return False


def _build_program():
    import concourse.bacc as bacc
    import concourse.tile as tile
    from concourse import bass, mybir

    f32 = mybir.dt.float32
    bf16 = mybir.dt.bfloat16
    i32 = mybir.dt.int32
    RELU = mybir.ActivationFunctionType.Relu

    nc = bacc.Bacc("TRN2", target_bir_lowering=False, debug=False)

    # idsT[p, g] = ids[g*128 + p] (indirect-DMA row offsets per 128-chunk)
    idx_ap = nc.dram_tensor("idsT", [128, NCH], i32, kind="ExternalInput").ap()
    web_ap = nc.dram_tensor("web", [V, H], bf16, kind="ExternalInput").ap()
    # w1ee packs [W1 | eeT]: [KG, MH + NE]
    w1ee_ap = nc.dram_tensor("w1ee", [KG, MH + NE], bf16, kind="ExternalInput").ap()
    b1c_ap = nc.dram_tensor("b1colT", [128, KCH], f32, kind="ExternalInput").ap()
    # w2p chunk-major: w2p[p, k*H+j] = W2[k*128+p, j] (rows 1000..1023 zero)
    w2_ap = nc.dram_tensor("w2p", [128, KCH * H], bf16, kind="ExternalInput").ap()
    # b2o [1, H+NE]: [b2 | ones] -> K=1 bias matmul
    b2o_ap = nc.dram_tensor("b2o", [1, H + NE], bf16, kind="ExternalInput").ap()
    maskT_ap = nc.dram_tensor("maskT", [NE, TOK], bf16, kind="ExternalInput").ap()
    # out[p, g*H:(g+1)*H] = token (g*128+p); host re-transposes
    out_ap = nc.dram_tensor("out", [128, NCH * H], bf16, kind="ExternalOutput").ap()

    with tile.TileContext(nc) as tc, ExitStack() as ctx:
        const = ctx.enter_context(tc.tile_pool(name="const", bufs=1))
        psA = ctx.enter_context(tc.tile_pool(name="psA", bufs=2, space="PSUM"))
        psE = ctx.enter_context(tc.tile_pool(name="psE", bufs=1, space="PSUM"))
        psC = ctx.enter_context(tc.tile_pool(name="psC", bufs=2, space="PSUM"))
        gpool = ctx.enter_context(tc.tile_pool(name="gath", bufs=NCH))

        # ---- loads -----------------------------------------------------------
        # sync HWDGE:   idx (gathers need it first) + w1ee + w2 1st half
        # scalar HWDGE: small weights + w2 2nd half + mask
        idx_sb = const.tile([128, NCH], i32)
        nc.sync.dma_start(idx_sb[:], idx_ap[:])
        w1ee_sb = const.tile([KG, MH + NE], bf16)
        nc.sync.dma_start(w1ee_sb[:], w1ee_ap[:])
        w1_sb = w1ee_sb[:, :MH]
        ee_sb = w1ee_sb[:, MH : MH + NE]
        b1_col = const.tile([128, KCH], f32)
        nc.scalar.dma_start(b1_col[:], b1c_ap[:])
        b2o_sb = const.tile([1, H + NE], bf16)
        nc.scalar.dma_start(b2o_sb[:], b2o_ap[:])
        # w2 in 2-chunk pieces alternating rings so stage2 k-chunks unblock
        # progressively instead of waiting for the whole 1.5MB under HBM
        # contention with the gather stream.
        w2_sb = const.tile([128, KCH * H], bf16)
        for piece in range(4):
            lo = piece * 2 * H
            hi = lo + 2 * H
            eng = nc.sync if piece % 2 == 0 else nc.scalar
            eng.dma_start(w2_sb[:, lo:hi], w2_ap[:, lo:hi])
        maskT_sb = const.tile([NE, TOK], bf16)
        nc.scalar.dma_start(maskT_sb[:], maskT_ap[:])

        # ---- vocab gathers: 16 x 128 rows via indirect DMA. Mainline-SWDGE
        # only: any dma_gather would force a ~13us Q7 library load during
        # which NO descriptors (even mainline) can be generated.
        gts = []
        for g in range(NCH):
            gt = gpool.tile([128, H], bf16, tag="gt")
            nc.gpsimd.indirect_dma_start(
                out=gt[:],
                out_offset=None,
                in_=web_ap[:],
                in_offset=bass.IndirectOffsetOnAxis(ap=idx_sb[:, g : g + 1], axis=0),
            )
            gts.append(gt)

        # ---- MLP stage 1: hT[k*128+p, e] = relu(W1.T @ ee.T + b1) -----------
        hT = const.tile([128, KCH, NE], bf16)
        nc.vector.memset(hT[96:128, KCH - 1, :], 0.0)
        for k in range(KCH):
            mw = 128 if k < KCH - 1 else MH - 128 * (KCH - 1)  # 104 in last
            ps = psA.tile([128, NE], f32, tag="ps")
            nc.tensor.matmul(
                out=ps[:mw, :],
                lhsT=w1_sb[:, k * 128 : k * 128 + mw],
                rhs=ee_sb[:],
                start=True,
                stop=True,
            )
            nc.scalar.activation(
                out=hT[:mw, k, :],
                in_=ps[:mw, :],
                func=RELU,
                bias=b1_col[:mw, k : k + 1],
            )

        # ---- MLP stage 2: ent = hT.T @ W2 + b2 ------------------------------
        # b2 enters the PSUM accumulation as a K=1 matmul of ones.T @ b2.
        entp = psE.tile([NE, H], f32)
        NGROUPS = ((0, 512), (512, H))
        for n0, n1 in NGROUPS:
            nc.tensor.matmul(
                out=entp[:, n0:n1],
                lhsT=b2o_sb[:, H : H + NE],
                rhs=b2o_sb[:, n0:n1],
                start=True,
                stop=False,
            )
        for k in range(KCH):
            for n0, n1 in NGROUPS:
                nc.tensor.matmul(
                    out=entp[:, n0:n1],
                    lhsT=hT[:, k, :],
                    rhs=w2_sb[:, k * H + n0 : k * H + n1],
                    start=False,
                    stop=(k == KCH - 1),
                )
        entS = const.tile([NE, H], bf16)
        nc.scalar.copy(entS[:], entp[:])

        # ---- main loop: scatter-matmul, add, grouped store ------------------
        # Vector drains every PSUM tile (measured best: a second drain lane
        # via scalar-ACT copies or gpsimd adds always loses to scheduler and
        # ring-order effects). Stores group 4 chunks; the last 4 ship as
        # 2-chunk stores to cut the drain tail.
        opool = ctx.enter_context(tc.tile_pool(name="outp", bufs=GQ))
        ot_q = None
        for g in range(NCH):
            if g % GCH == 0:
                ot_q = opool.tile([128, GCH, H], bf16, tag="ot")
            sc = psC.tile([128, H], f32, tag="sc")
            for n0, n1 in NGROUPS:
                nc.tensor.matmul(
                    out=sc[:, n0:n1],
                    lhsT=maskT_sb[:, g * 128 : (g + 1) * 128],
                    rhs=entS[:, n0:n1],
                    start=True,
                    stop=True,
                )
            nc.vector.tensor_add(ot_q[:, g % GCH, :], gts[g][:], sc[:])
            if g == 13 or g == 15:
                st_eng = nc.sync if g == 13 else nc.scalar
                st_eng.dma_start(
                    out_ap[:, (g - 1) * H : (g + 1) * H],
                    ot_q[:, (g - 1) % GCH : g % GCH + 1, :],
                )
            elif (g + 1) % GCH == 0:
                q = g // GCH
                st_eng = nc.sync if q % 2 == 0 else nc.scalar
                st_eng.dma_start(
                    out_ap[:, q * GCH * H : (q + 1) * GCH * H], ot_q[:]
                )

    nc.compile()
    return nc


def _get_program():
    global _PROGRAM
    if _PROGRAM is None:
        _PROGRAM = _build_program()
    return _PROGRAM


def _prep_shards(inputs):
    import ml_dtypes

    bf = ml_dtypes.bfloat16
    ids = np.asarray(inputs["input_ids"]).astype(np.int32)
    ee = np.asarray(inputs["entity_embeddings"], dtype=np.float32)
    mask = np.asarray(inputs["entity_mask"], dtype=np.float32)
    we = np.asarray(inputs["word_embedding"], dtype=np.float32)
    W1 = np.asarray(inputs["W1"], dtype=np.float32)
    b1 = np.asarray(inputs["b1"], dtype=np.float32)
    W2 = np.asarray(inputs["W2"], dtype=np.float32)
    b2 = np.asarray(inputs["b2"], dtype=np.float32)

    web = np.ascontiguousarray(we.astype(bf))
    w1b = W1.astype(bf)
    w2_pad = np.concatenate([W2, np.zeros((KCH * 128 - MH, H), np.float32)], 0)
    w2p = np.ascontiguousarray(
        w2_pad.reshape(KCH, 128, H).transpose(1, 0, 2).reshape(128, KCH * H).astype(bf)
    )
    b2o = np.ascontiguousarray(
        np.concatenate([b2[None, :], np.ones((1, NE), np.float32)], 1).astype(bf)
    )  # [1, H+NE]
    b1pad = np.concatenate([b1, np.zeros(KCH * 128 - MH, np.float32)])
    b1colT = np.ascontiguousarray(b1pad.reshape(KCH, 128).T)  # [128, KCH]

    in_maps = []
    for i in range(NCORES):
        sl = slice(BPC * i, BPC * (i + 1))
        ids_c = ids[sl].reshape(-1)  # [TOK]
        idsT = np.ascontiguousarray(ids_c.reshape(NCH, 128).T)  # [128, NCH]
        # tail tokens (chunks 10..15) wrapped mod 16 for dma_gather:
        # idx16t[p, s] = ids_c[1280 + s*16 + (p % 16)]
        tail = ids_c[10 * 128 :].reshape(-1, 16)  # [48, 16]
        idx16t = np.tile(tail.T, (8, 1)).astype(np.int16)  # [128, 48]
        eeT = ee[sl].reshape(NE, KG).T.astype(bf)  # [KG, NE]
        w1ee = np.ascontiguousarray(np.concatenate([w1b, eeT], 1))
        # block-diagonal [NE, TOK] mask (0/1 values: exact in bf16)
        maskT = np.zeros((NE, TOK), np.float32)
        for b in range(BPC):
            maskT[b * E : (b + 1) * E, b * S : (b + 1) * S] = mask[BPC * i + b]
        in_maps.append(
            {
                "idsT": idsT,
                "idx16t": np.ascontiguousarray(idx16t),
                "web": web,
                "w1ee": w1ee,
                "b1colT": b1colT,
                "w2p": w2p,
                "b2o": b2o,
                "maskT": np.ascontiguousarray(maskT.astype(bf)),
            }
        )
    return in_maps


def kernel(**inputs) -> np.ndarray:
    from concourse.bass_utils import run_bass_kernel_spmd

    trace = _maybe_enable_profiling()
    nc = _get_program()
    in_maps = _prep_shards(inputs)
    res = run_bass_kernel_spmd(
        nc, in_maps, core_ids=list(range(NCORES)), trace=trace
    )
    if trace and res.exec_time_ns is not None:
        print(f"HW exec time: {res.exec_time_ns} ns")
    # out[p, g*H:(g+1)*H] = token (g*128+p): re-transpose per core
    outs = []
    for i in range(NCORES):
        o = np.asarray(res.results[i]["out"]).astype(np.float32)
        o = o.reshape(128, NCH, H).transpose(1, 0, 2).reshape(BPC, S, H)
        outs.append(o)
    return np.concatenate(outs, 0)


if __name__ == "__main__":
    rng = np.random.default_rng(0)
    inputs = {
        "input_ids": rng.integers(0, V, (B, S)).astype(np.int32),
        "entity_embeddings": rng.standard_normal((B, E, KG), dtype=np.float32),
        "entity_mask": (rng.random((B, E, S)) < 0.02).astype(np.float32),
        "word_embedding": rng.standard_normal((V, H), dtype=np.float32) * 0.02,
        "W1": rng.standard_normal((KG, MH), dtype=np.float32) * 0.02,
        "b1": np.zeros(MH, np.float32),
        "W2": rng.standard_normal((MH, H), dtype=np.float32) * 0.02,
        "b2": np.zeros(H, np.float32),
    }
    out = kernel(**inputs)
    ref = inputs["word_embedding"][inputs["input_ids"]] + np.einsum(
        "bes,beh->bsh",
        inputs["entity_mask"],
        np.maximum(
            inputs["entity_embeddings"] @ inputs["W1"] + inputs["b1"], 0.0
        )
        @ inputs["W2"]
        + inputs["b2"],
    )
    err = np.abs(out - ref).max() / max(np.abs(ref).max(), 1e-12)
    print("self-check rel err:", err)
